# revision 1
# baseline (speedup 1.0000x reference)
"""Trainium2 Bass kernel for CrispComposition.

Computes out[b, i] = max_o( min(m[b, i], weight[i, o]) ).

Since min(m, .) is monotone non-decreasing, the max over o commutes with it:
    max_o min(m, w[i, o]) = min(m, max_o w[i, o])
which is bit-exact in floating point (both sides select one of the original
values, no arithmetic). So the kernel reduces weight over its OUT axis once
(wmax[i] = max_o weight[i, o]) and streams an elementwise min over m.

Sharding: data-parallel on the batch axis of m across 8 NeuronCores; weight is
replicated and each core computes wmax locally.

Note: HWDGE DMAs round-robin over 8 completion-semaphore lanes and a DMA
instruction only supports a single sync wait, so the kernel is structured to
issue at most 8 HWDGE DMAs with at most one data dependency each.
"""

import numpy as np

import concourse.bacc as bacc
import concourse.mybir as mybir
from concourse.bass_utils import run_bass_kernel_spmd
from concourse.masks import make_identity
from concourse.tile import TileContext

B, IN, OUT = 4096, 512, 256
NCORES = 8
BS = B // NCORES  # 512 batch rows per core
P = 128  # SBUF partitions

F32 = mybir.dt.float32


def build_bass(
    repeat=1,
    n_split=4,
    store_engine="sync",
    bufs=4,
    single_transpose=True,
    min_engines="dve",
    bcast_from_psum=True,
    load_engine="sync",
):
    nc = bacc.Bacc()
    m_in = nc.declare_dram_parameter("m", [BS, IN], F32, isOutput=False)
    w_in = nc.declare_dram_parameter("weight", [IN, OUT], F32, isOutput=False)
    out = nc.declare_dram_parameter("out", [BS, IN], F32, isOutput=True)

    n_wt = IN // P  # 4 column-blocks of wmax
    rows_half = BS // n_split
    n_sub = rows_half // P  # row-groups per partition per tile

    with TileContext(nc) as tc:
        with (
            tc.tile_pool(name="consts", bufs=1) as consts,
            tc.tile_pool(name="wpool", bufs=n_wt) as wpool,
            tc.tile_pool(name="mpool", bufs=bufs) as mpool,
            tc.tile_pool(name="opool", bufs=bufs) as opool,
            tc.tile_pool(name="psum", bufs=1, space="PSUM") as psum,
        ):
            # ---- wmax[i] = max_o weight[i, o] ----
            # 4 independent load+reduce pairs so the first reduce starts as
            # soon as the first 128 weight rows land.
            wmax4 = consts.tile([P, n_wt], F32)
            for t in range(n_wt):
                wt = wpool.tile([P, OUT], F32, tag="w")
                nc.sync.dma_start(out=wt, in_=w_in[t * P : (t + 1) * P, :])
                nc.vector.reduce_max(
                    out=wmax4[:, t : t + 1], in_=wt, axis=mybir.AxisListType.X
                )

            ones = consts.tile([P, P], F32)
            nc.gpsimd.memset(ones, 1.0)
            identity = consts.tile([P, P], F32)
            make_identity(nc, identity)

            # bcast[q, 128t+p] = wmax4[p, t] for every partition q, per block:
            #   diag_t = identity * wmax4[:, t]   (per-partition scalar mul)
            #   bc[:, t*128:(t+1)*128] = ones^T @ diag_t
            # Sums of one nonzero value are exact, so this is bit-exact.
            bc_ps = psum.tile([P, IN], F32, tag="bc")
            bcast = consts.tile([P, IN], F32)
            if single_transpose:
                for t in range(n_wt):
                    diag = consts.tile([P, P], F32, tag=f"diag{t}")
                    nc.vector.tensor_scalar_mul(diag, identity, wmax4[:, t : t + 1])
                    nc.tensor.matmul(
                        bc_ps[:, t * P : (t + 1) * P],
                        lhsT=ones,
                        rhs=diag,
                        start=True,
                        stop=True,
                    )
            else:
                for t in range(n_wt):
                    row_ps = psum.tile([1, P], F32, tag="row")
                    nc.tensor.transpose(row_ps, wmax4[:, t : t + 1], identity)
                    row = consts.tile([1, P], F32, tag=f"row{t}")
                    nc.vector.tensor_copy(out=row, in_=row_ps)
                    nc.tensor.matmul(
                        bc_ps[:, t * P : (t + 1) * P],
                        lhsT=ones[0:1, :],
                        rhs=row,
                        start=True,
                        stop=True,
                    )
            if bcast_from_psum:
                bcast = bc_ps  # mins read the PSUM bank directly
            else:
                nc.vector.tensor_copy(out=bcast, in_=bc_ps)

            # ---- main stream: out = min(m, bcast) ----
            store_dma = nc.scalar if store_engine == "scalar" else nc.sync
            load_dma = nc.scalar if load_engine == "scalar" else nc.sync
            has_pool_tt = hasattr(nc.gpsimd, "tensor_tensor")
            for _ in range(repeat):
                for h in range(n_split):
                    lo, hi = h * rows_half, (h + 1) * rows_half
                    mt = mpool.tile([P, n_sub, IN], F32, tag="m")
                    load_dma.dma_start(
                        out=mt, in_=m_in[lo:hi].rearrange("(p n) d -> p n d", n=n_sub)
                    )
                    ot = opool.tile([P, n_sub, IN], F32, tag="o")
                    for n in range(n_sub):
                        idx = h * n_sub + n
                        eng = nc.vector
                        if min_engines == "mixed" and has_pool_tt and idx % 2 == 1:
                            eng = nc.gpsimd
                        eng.tensor_tensor(
                            out=ot[:, n, :],
                            in0=mt[:, n, :],
                            in1=bcast,
                            op=mybir.AluOpType.min,
                        )
                    store_dma.dma_start(
                        out=out[lo:hi].rearrange("(p n) d -> p n d", n=n_sub), in_=ot
                    )

    return nc


_NC_CACHE = None


def _get_nc():
    global _NC_CACHE
    if _NC_CACHE is None:
        nc = build_bass()
        # Run Bacc's legalization (sync-wait splitting, register allocation)
        # before the PJRT path serializes the module.
        nc.finalize()
        _NC_CACHE = nc
    return _NC_CACHE


def run(m, weight, **spmd_kwargs):
    """Run the bass kernel; returns (full_output, BassKernelResults)."""
    m = np.ascontiguousarray(m, dtype=np.float32)
    weight = np.ascontiguousarray(weight, dtype=np.float32)
    nc = _get_nc()
    in_maps = [
        {"m": m[c * BS : (c + 1) * BS], "weight": weight} for c in range(NCORES)
    ]
    res = run_bass_kernel_spmd(nc, in_maps, list(range(NCORES)), **spmd_kwargs)
    full = np.concatenate(
        [np.asarray(res.results[c]["out"]) for c in range(NCORES)], axis=0
    )
    return full.astype(np.float32, copy=False), res


def kernel(m, weight):
    return run(m, weight)[0]

